# revision 5
# baseline (speedup 1.0000x reference)
"""Masked attention-aggregator kernel for Trainium2 (8 NeuronCores, SPMD).

Reference computation (B=16, N=2048, D=128, DQ=64), all fp32:
    q = x @ Wq.T + bq                      [B, N, DQ]
    k = x @ Wk.T + bk                      [B, N, DQ]
    s = (k @ q.T) / sqrt(DQ)               [B, N, N]
    w = softmax(s + (mask[m]>0 ? 0 : -1e9), axis=m)
    out = w @ x                            [B, N, D]

Strategy: data-parallel over batch (2 batches per core).  Per batch, a
flash-style streaming attention that never materializes [N, N]; the host
permutes the n axis so mask-kept columns come first (the computation is
n-equivariant; the host un-permutes the output).  Dropped columns are
killed exactly by a -192 additive penalty delivered through the exp
instruction's per-partition bias, driving their softmax weight to 0.

Engine split (found by HW bisection -- GPSIMD/Pool cannot touch PSUM on
real TRN2, and Pool tensor ops / DVE int16 converts are far slower on HW
than the cost model suggests):
  PE   : one z-projection z = (Wq^T Wk/sqrt(DQ))^T x per batch (the two
         reference projections collapse into a single host-precomputed
         [D,D] matrix M, so scores are z_chunk^T @ xt directly -- no k
         projection, no k evacuation, no ones rows), scores ST[m,n]
         (f16, PSUM f32), numerator out^T[d,n] += xcb_chunk^T @ E^T,
         per-group den matmul ones^T @ (eacc + et_last) in PSUM.
  ACT  : all exp tiles (fp32 PSUM -> f16 E); the mask penalty AND the
         bk.q[m] bias term ride the exp's per-partition bias AP (host-
         computed penb); the z evacuation; exp table preloaded early.
         (bq.k[n] and bq.bk bias terms are constant along the softmax
         axis and cancel exactly.)
  DVE  : den accumulation (f16 2x adds), reciprocal_approx_fast, final
         divide (oa PSUM x rden -> f16).
  Pool : software-DGE DMA dispatch only (pen/ones row loads, stores).

HW-derived layout rules: dram transfers must be large and contiguous
(xt is one 512KB DMA; xcb is pre-transposed m-chunk-major on the host,
removing the on-device xbar transpose; the output is stored group-major
[BPC, ngroups, D, NG] so every store is contiguous).  Tiles are split
along span boundaries (qt main/tail, kt per group, last-chunk E halves)
because the Tile dependency tracker is per-tile, not per-range.  The
per-group den matmul folds the last chunk via PSUM accumulation so the
serial eacc add chain never gates it.  All tensor data travels as f16
(PSUM accumulation in f32); measured rel err ~5.4e-4 vs the 2e-2 gate.
"""

import math
import os

import numpy as np

B, N, D, DQ = 16, 2048, 128, 64
NCORES = 8
BPC = B // NCORES  # batches per core
NG = 1024          # n-group size

A16 = 1024.0 * 1.4426950408889634        # f16 Schraudolph multiplier
PEN_KEEP = float(np.float16(15360.0 / A16))   # = B/A in f16 (10.3984375)
PEN_DROP = PEN_KEEP - 192.0

_cache = {}
INST_LABELS = {}


def _lab(inst, label):
    try:
        INST_LABELS[inst.ins.name] = label
    except AttributeError:
        try:
            INST_LABELS[inst.name] = label
        except Exception:
            pass
    return inst


def _calibrate_act_bias():
    """ACT tiles must match the Schraudolph tiles' implied scale alpha."""
    s = np.random.default_rng(0).normal(0.0, 1.0 / 3.0, 100000).astype(np.float32)
    t = np.maximum((s + np.float32(PEN_KEEP)) * np.float32(A16), 0.0)
    bits = t.astype(np.int16)
    e = bits.view(np.float16).astype(np.float64)
    ln_alpha = np.mean(np.log(e) - s.astype(np.float64))
    return float(ln_alpha - PEN_KEEP)


ACT_EXP_BIAS = _calibrate_act_bias()

# per-m-chunk engine assignment (mc=9):  A=ACT exact exp, P=Pool, V=DVE
# Schraudolph.  Keep Schraudolph tiles a minority for accuracy.
EXP_ENG = ['A'] * 16
# den accumulation engine per m-chunk (m>=1 adds; m==0 is the eacc init)
DEN_ENG = ['-'] + ['V'] * 15


def _build_program(mcp: int, reps: int = 1, has_bias: bool = False):
    """Per-core Bass program for a compacted/padded m-size of mcp."""
    import concourse.bass as bass
    import concourse.tile as tile
    from concourse import bacc, mybir

    f32 = mybir.dt.float32
    f16 = mybir.dt.float16
    i16 = mybir.dt.int16
    mc = mcp // 128
    ngroups = N // NG
    assert len(EXP_ENG) >= mc and len(DEN_ENG) >= mc

    nc = bacc.Bacc("TRN2", target_bir_lowering=False, debug=False, num_devices=1)

    xt = nc.dram_tensor("xt", [BPC, D, N], f16, kind="ExternalInput").ap()
    xcb = nc.dram_tensor("xcb", [BPC, 128, mc * D], f16, kind="ExternalInput").ap()
    penb = nc.dram_tensor("penb", [BPC, 128, mc], f32, kind="ExternalInput").ap()
    mw = nc.dram_tensor("mw", [D, D], f16, kind="ExternalInput").ap()
    out = nc.dram_tensor("out", [BPC, N // NG, D, NG], f16,
                         kind="ExternalOutput").ap()

    with tile.TileContext(nc) as tc:
        with (
            tc.tile_pool(name="singles", bufs=1) as singles,
            tc.tile_pool(name="xtp", bufs=2) as xtp,
            tc.tile_pool(name="xcp", bufs=2) as xcp,
            tc.tile_pool(name="qtp", bufs=2) as qtp,
            tc.tile_pool(name="ktp", bufs=2) as ktp,
            tc.tile_pool(name="etp", bufs=6) as etp,
            tc.tile_pool(name="eap", bufs=2) as eap,
            tc.tile_pool(name="rdp", bufs=4) as rdp,
            tc.tile_pool(name="nrmp", bufs=4) as nrmp,
            tc.tile_pool(name="st", bufs=3, space="PSUM") as stp,
            tc.tile_pool(name="oa", bufs=1, space="PSUM") as oap,
        ):
            mw_sb = singles.tile([D, D], f16)
            nc.sync.dma_start(mw_sb[:], mw[:])
            ones = singles.tile([128, 128], f16)
            nc.vector.memset(ones[:], 1.0)
            zb = singles.tile([128, 1], f32)
            nc.vector.memset(zb[:], 0.0)
            # warm the ACT exp table while the first loads are in flight
            dummy = singles.tile([128, 1], f16)
            nc.scalar.activation(dummy[:], zb[:],
                                 mybir.ActivationFunctionType.Exp)


            def body():
              xts, xcbs, qts, pens = [], [], [], []
              for b in range(BPC):
                # ---- loads (all batches up front; SP queue is FIFO) ----
                xt_t = xtp.tile([D, N], f16, tag="xt")
                nc.sync.dma_start(xt_t[:], xt[b][:])
                penb_t = ktp.tile([128, mc], f32, tag="penb", name="penb_t")
                nc.gpsimd.dma_start(penb_t[:], penb[b])
                xcb_t = xcp.tile([128, mc * D], f16, tag="xc")
                nc.sync.dma_start(xcb_t[:], xcb[b][:])
                zt_m = qtp.tile([D, min(NG, mcp)], f16, tag="qt", name="zt_m")
                zt_x = None
                if mcp > NG:
                    zt_x = qtp.tile([D, mcp - NG], f16, tag="qtx", name="zt_x")
                xts.append(xt_t); xcbs.append(xcb_t)
                qts.append((zt_m, zt_x)); pens.append(penb_t)
              for b in range(BPC):
                xt_t, xcb_t = xts[b], xcbs[b]
                (zt_m, zt_x), penb_t = qts[b], pens[b]

                # ---- z-projection: z = (Wq^T Wk / 8)^T x, so scores are
                # z_chunk^T @ xt directly (no k projection, no kt tiles) ----
                def project_span(dst, j0, span):
                    pp = stp.tile([128, NG], f32, tag="st")
                    for j in range(0, span, 512):
                        jw = min(512, span - j)
                        _lab(nc.tensor.matmul(
                            pp[:, j:j + jw], mw_sb[:],
                            xt_t[:, j0 + j:j0 + j + jw],
                            start=True, stop=True), f"b{b}.proj0.j{j0+j}")
                    _lab(nc.scalar.activation(
                        dst[:, 0:span], pp[:, 0:span],
                        mybir.ActivationFunctionType.Copy),
                         f"b{b}.evac0.j{j0}")

                project_span(zt_m, 0, min(NG, mcp))
                if mcp > NG:
                    project_span(zt_x, NG, mcp - NG)

                # ---- attention over n-groups ----
                # Flat software pipeline across groups: scores/exp for item
                # i+1 are emitted before the numerator matmul of item i, so
                # the next group's scores can hide the last chunk's exp
                # latency at every group boundary.
                state = {}
                def new_group(g):
                    oa = oap.tile([128, NG], f32, tag="oa")
                    eacc = eap.tile([128, NG], f16, tag="eacc")
                    ets = [None] * mc
                    state[g] = (oa, eacc, ets)

                def scores_exp(g, m):
                    oa, eacc, ets = state[g]
                    st = stp.tile([128, NG], f32, tag="st")
                    qsrc = zt_m if m * 128 < NG else zt_x
                    qoff = m * 128 if m * 128 < NG else m * 128 - NG
                    for h in range(NG // 512):
                        n0 = g * NG + h * 512
                        _lab(nc.tensor.matmul(
                            st[:, h * 512:(h + 1) * 512],
                            qsrc[:, qoff:qoff + 128],
                            xt_t[:, n0:n0 + 512],
                            start=True, stop=True), f"b{b}.g{g}.S{m}.h{h}")
                    if m == mc - 1:
                        eth0 = etp.tile([128, NG // 2], f16, tag="et", name="eth0")
                        eth1 = etp.tile([128, NG // 2], f16, tag="et", name="eth1")
                        et = [eth0, eth1]
                    else:
                        et = eacc if m == 0 else etp.tile([128, NG], f16, tag="et")
                    ets[m] = et
                    parts = et if isinstance(et, list) else [et]
                    hsplit = len(parts)
                    for eh, ep in enumerate(parts):
                        es = slice(eh * (NG // hsplit), (eh + 1) * (NG // hsplit))
                        if EXP_ENG[m] == 'A':
                            _lab(nc.scalar.activation(
                                ep[:], st[:, es],
                                mybir.ActivationFunctionType.Exp,
                                bias=penb_t[:, m:m + 1]), f"b{b}.g{g}.E{m}.{eh}")
                        else:
                            _lab(nc.vector.tensor_scalar(
                                ep[:].bitcast(i16), st[:, es], A16, 0.0,
                                mybir.AluOpType.mult, mybir.AluOpType.max),
                                 f"b{b}.g{g}.E{m}.{eh}")

                def numer_den(g, m):
                    oa, eacc, ets = state[g]
                    first, last = (m == 0), (m == mc - 1)
                    et = ets[m]
                    for h in range(NG // 512):
                        hs = slice(h * 512, (h + 1) * 512)
                        rhs = et[h][:] if isinstance(et, list) else et[:, hs]
                        _lab(nc.tensor.matmul(oa[:, hs],
                                         xcb_t[:, m * D:(m + 1) * D],
                                         rhs, start=first, stop=last),
                             f"b{b}.g{g}.N{m}.h{h}")
                    if not first and not last:
                        deng = nc.vector if DEN_ENG[m] == 'V' else nc.gpsimd
                        _lab(deng.tensor_add(eacc[:], eacc[:], et[:]),
                             f"b{b}.g{g}.D{m}")

                def finish_group(g):
                    oa, eacc, ets = state[g]
                    # den = ones.T @ (eacc + et_last), accumulated in PSUM so
                    # the last chunk's E never enters the serial eacc chain
                    dn = stp.tile([128, NG], f32, tag="st")
                    for h in range(NG // 512):
                        hs = slice(h * 512, (h + 1) * 512)
                        _lab(nc.tensor.matmul(dn[:, hs], ones[:], eacc[:, hs],
                                         start=True, stop=False), f"b{b}.g{g}.dn{h}")
                        _lab(nc.tensor.matmul(dn[:, hs], ones[:], ets[mc - 1][h][:],
                                         start=False, stop=True), f"b{b}.g{g}.dn{h}b")
                    nmf = nrmp.tile([128, NG], f16, tag="nrm", name="nmf")
                    rdf = rdp.tile([128, NG], f32, tag="rden", name="rdf")
                    _lab(nc.vector.reciprocal_approx_fast(rdf[:], dn[:]),
                         f"b{b}.g{g}.rcp")
                    _lab(nc.vector.tensor_mul(nmf[:], oa[:], rdf[:]),
                         f"b{b}.g{g}.nrm")
                    _lab(nc.gpsimd.dma_start(out[b][g], nmf[:]),
                         f"b{b}.g{g}.store")

                items = [(g, m) for g in range(ngroups) for m in range(mc)]
                new_group(0)
                scores_exp(*items[0])
                for i in range(1, len(items)):
                    g, m = items[i]
                    if m == 0:
                        new_group(g)
                    scores_exp(g, m)
                    pg, pm = items[i - 1]
                    numer_den(pg, pm)
                    if pm == mc - 1:
                        finish_group(pg)
                numer_den(*items[-1])
                finish_group(items[-1][0])

            if reps > 1:
                with tc.For_i(0, reps, 1):
                    body()
            else:
                body()

    nc.compile()
    return nc


def _prep(x, mask, Wq, bq, Wk, bk):
    """Host-side prep: n-permutation (kept cols first), f16 casts, sharding."""
    x = np.asarray(x, dtype=np.float32)
    mask = np.asarray(mask)
    Wq = np.asarray(Wq, dtype=np.float32)
    bq = np.asarray(bq, dtype=np.float32)
    Wk = np.asarray(Wk, dtype=np.float32)
    bk = np.asarray(bk, dtype=np.float32)

    scale = np.float32(1.0 / math.sqrt(DQ))

    perm = np.empty((B, N), dtype=np.int64)
    counts = []
    for b in range(B):
        keep = np.nonzero(mask[b] > 0)[0]
        drop = np.nonzero(mask[b] <= 0)[0]
        perm[b, :len(keep)] = keep
        perm[b, len(keep):] = drop
        counts.append(len(keep))
    mcap = max(max(counts), 1)
    mcp = ((mcap + 127) // 128) * 128
    mc = mcp // 128

    # x with columns permuted (kept first), transposed to [D, N], f16
    xp = np.take_along_axis(x, perm[:, :, None], axis=1)       # [B, N, D]
    xt = np.ascontiguousarray(xp.transpose(0, 2, 1)).astype(np.float16)

    # m-chunk-major compacted x (host transpose): [128, mc*D]
    xcb = np.ascontiguousarray(
        xp[:, :mcp].reshape(B, mc, 128, D).transpose(0, 2, 1, 3)
    ).reshape(B, 128, mc * D).astype(np.float16)

    # per-chunk exp bias: 0 for kept m, -192 for dropped/padded m
    penb = np.full((B, 128, mc), -192.0, dtype=np.float32)
    m = np.arange(mcp)
    w_v = (scale * (Wq.T @ bk)).astype(np.float32)  # [D]
    for b in range(B):
        v = xp[b, :mcp].astype(np.float32) @ w_v   # bk . q_m * scale
        penb[b] = np.where(m < counts[b], v, -192.0).reshape(mc, 128).T

    # scores = x^T (Wq^T Wk * scale) x ; fold the 1/sqrt(DQ) into M.
    # Biases: bq.k[n]*scale and bq.bk*scale are constant along the softmax
    # (m) axis and cancel; bk.q[m]*scale is a per-m additive computed on the
    # host and folded into the exp bias below.
    has_bias = False
    mw = (Wq.T @ Wk * scale).astype(np.float16)   # [D, D]

    in_maps = []
    for c in range(NCORES):
        s = slice(c * BPC, (c + 1) * BPC)
        in_maps.append({
            "xt": xt[s], "xcb": xcb[s], "penb": penb[s], "mw": mw,
        })
    return in_maps, mcp, perm, has_bias


def kernel(x, mask, Wq, bq, Wk, bk):
    from concourse import bass_utils

    in_maps, mcp, perm, has_bias = _prep(x, mask, Wq, bq, Wk, bk)

    key = (mcp, has_bias)
    if key not in _cache:
        _cache[key] = _build_program(mcp, has_bias=has_bias)
    nc = _cache[key]

    res = bass_utils.run_bass_kernel_spmd(
        nc, in_maps, core_ids=list(range(NCORES)),
        trace=bool(os.environ.get("BASS_TRACE")),
    )
    kernel._last_results = res

    out_t = np.concatenate([res.results[c]["out"] for c in range(NCORES)], axis=0)
    out_t = out_t.transpose(0, 2, 1, 3).reshape(B, D, N)  # [B, D, N]
    outp = out_t.astype(np.float32).transpose(0, 2, 1)  # [B, N, D], permuted n
    out = np.empty_like(outp)
    bidx = np.arange(B)[:, None]
    out[bidx, perm] = outp
    return np.ascontiguousarray(out)
